# revision 1
# baseline (speedup 1.0000x reference)
"""Biaffine labeler kernel for 8x Trainium2 NeuronCores.

Full-input contract: kernel(**inputs) takes the unsharded inputs and
returns the full [8, 256, 50] float32 logits.

Sharding: data-parallel over B — core i handles batch i. The projection
weights, bilinear tensor W, and biases are replicated (staged identically
for every core).

Per-core pipeline (T=256 tokens, D=1024, DL=512, NL=50 labels):
  1. head_label = head[b] @ Whead                 (PE, bf16, K=1024)
  2. one-hot(idx) built on DVE; sel^T = head_label^T gathered via a
     one-hot matmul on PE; bhead added on ACT during PSUM->SBUF copy
  3. dep_label  = dep[b] @ Wdep + bdep            (PE bf16 + DVE add)
  4. P_n = sel @ W[n]^T for all 50 labels         (PE, bf16, 400 matmuls)
  5. logits[t,n] = sum_d dep_label[t,d] * P_n[t,d]
     via DVE scalar_tensor_tensor accum_out; label bias added at the end.

W streams from HBM in bf16 on the single sync-engine DMA queue, in
label-group chunks with ramped sizes (1,2,3,4,4,...) so the first groups
arrive early; the stream is software-pipelined 3 deep. DMA dispatches
cost ~0.65us each on the sequencer, so inputs are packed into a few
large transfers ordered just-in-time.
"""

import sys

sys.path.insert(0, "/opt/trn_rl_repo")

import numpy as np
import ml_dtypes

B, T, D = 8, 256, 1024
NL, DL = 50, 512
GROUP_SIZES = [1, 1, 2, 2] + [4] * 11  # sums to 50
N_GROUPS = len(GROUP_SIZES)
GF = 4 * 4 * DL  # max free-dim elems per W group chunk (8192)

# constpack layout (f32 columns)
C_IDX = 0          # [128, 256] idx broadcast
C_BDEP = 256       # [128, 512] bdep broadcast
C_BIAS = 768       # [128, 50] label bias broadcast
C_IOTA = 818       # [128, 2] iota columns
C_BHEAD = 820      # [128, 4] bhead chunks
C_TOT = 824

BF16 = ml_dtypes.bfloat16

# Stash of the last run's BassKernelResults (exec_time_ns when BASS_TRACE=1).
LAST_RESULTS = None

_NC_CACHE = None


def _group_ranges():
    out = []
    n0 = 0
    for sz in GROUP_SIZES:
        out.append((n0, n0 + sz))
        n0 += sz
    return out


def _build_nc():
    import concourse.bacc as bacc
    import concourse.mybir as mybir
    import concourse.tile as tile

    bf = mybir.dt.bfloat16
    f32 = mybir.dt.float32
    Alu = mybir.AluOpType
    Act = mybir.ActivationFunctionType

    nc = bacc.Bacc(None)

    # --- DRAM I/O ---------------------------------------------------------
    # headpack: [whead k-slabs 0-3 | headT k-slabs 0-3 | whead 4-7 | headT 4-7]
    headpack = nc.dram_tensor("headpack", [128, 6144], bf, kind="ExternalInput")
    deppack = nc.dram_tensor("deppack", [128, 6144], bf, kind="ExternalInput")
    constpack = nc.dram_tensor("constpack", [128, C_TOT], f32, kind="ExternalInput")
    wg = nc.dram_tensor("wg", [N_GROUPS, 128, GF], bf, kind="ExternalInput")
    out = nc.dram_tensor("out", [256, 64], f32, kind="ExternalOutput")

    ranges = _group_ranges()

    with tile.TileContext(nc) as tc:
        with (
            tc.sbuf_pool(name="cpool", bufs=1) as cpool,
            tc.sbuf_pool(name="persist", bufs=1) as pers,
            tc.sbuf_pool(name="wpool", bufs=4) as wpool,
            tc.sbuf_pool(name="spool", bufs=4) as spool,
            tc.psum_pool(name="ps", bufs=8) as ps,
        ):
            # --- input DMAs, just-in-time order ---------------------------
            hp_sb = cpool.tile([128, 6144], bf)
            nc.sync.dma_start(hp_sb[:, :3072], headpack[:, :3072])
            nc.sync.dma_start(hp_sb[:, 3072:], headpack[:, 3072:])
            cp_sb = cpool.tile([128, C_TOT], f32)
            nc.sync.dma_start(cp_sb[:], constpack[:])

            wg_tiles = {}

            def issue_wg(g):
                ng = ranges[g][1] - ranges[g][0]
                wt = wpool.tile([128, GF], bf, tag="wg", name=f"wg{g}")
                nc.sync.dma_start(wt[:, : ng * 4 * 512], wg[g, :, : ng * 4 * 512])
                wg_tiles[g] = wt

            issue_wg(0)

            dp_sb = cpool.tile([128, 6144], bf)
            nc.sync.dma_start(dp_sb[:, :3072], deppack[:, :3072])
            nc.sync.dma_start(dp_sb[:, 3072:], deppack[:, 3072:])

            issue_wg(1)
            issue_wg(2)
            issue_wg(3)

            # headpack slab views: whead k-slab k at half h=(k//4):
            #   whead slab k -> hp[:, h*3072 + (k%4)*512 : +512]
            #   headT slab k -> hp[:, h*3072 + 2048 + (k%4)*256 : +256]
            def whead_slab(sb, k):
                h = k // 4
                o = h * 3072 + (k % 4) * 512
                return sb[:, o : o + 512]

            def headT_slab(sb, k, j):
                h = k // 4
                o = h * 3072 + 2048 + (k % 4) * 256 + j * 128
                return sb[:, o : o + 128]

            # --- head projection: head_label[j] = [128 t, 512 d] bf16 -----
            head_label = []
            for j in range(2):
                ph = ps.tile([128, 512], f32, tag="ps", name=f"ph{j}")
                for k in range(8):
                    nc.tensor.matmul(
                        ph[:],
                        lhsT=headT_slab(hp_sb, k, j),
                        rhs=whead_slab(hp_sb, k),
                        start=(k == 0),
                        stop=(k == 7),
                    )
                hlj = pers.tile([128, 512], bf, tag=f"hl{j}", name=f"hl{j}")
                nc.scalar.copy(hlj[:], ph[:])
                head_label.append(hlj)

            # --- one-hot of head_indices: oh[j][p, t] = (idx[t] == 128j+p)
            onehot = []
            for j in range(2):
                ohj = pers.tile([128, 256], bf, tag=f"oh{j}", name=f"oh{j}")
                nc.vector.tensor_scalar(
                    out=ohj[:],
                    in0=cp_sb[:, C_IDX : C_IDX + 256],
                    scalar1=cp_sb[:, C_IOTA + j : C_IOTA + j + 1],
                    scalar2=None,
                    op0=Alu.is_equal,
                )
                onehot.append(ohj)

            # --- gather: selT[c] = [128 e, 256 t] bf16 (+bhead on copy) ---
            selT = []
            for c in range(4):
                pg = ps.tile([128, 256], f32, tag="ps", name=f"pg{c}")
                for j in range(2):
                    nc.tensor.matmul(
                        pg[:],
                        lhsT=head_label[j][:, c * 128 : (c + 1) * 128],
                        rhs=onehot[j][:],
                        start=(j == 0),
                        stop=(j == 1),
                    )
                sc = pers.tile([128, 256], bf, tag=f"sel{c}", name=f"sel{c}")
                nc.scalar.activation(
                    sc[:],
                    pg[:],
                    Act.Identity,
                    bias=cp_sb[:, C_BHEAD + c : C_BHEAD + c + 1],
                    scale=1.0,
                )
                selT.append(sc)

            # --- output accumulators --------------------------------------
            out_sb = []
            for m in range(2):
                om = pers.tile([128, 64], f32, tag=f"out{m}", name=f"out{m}")
                out_sb.append(om)

            # --- main-loop group body -------------------------------------
            # k-inner so each PSUM bank completes early; per-group W chunk
            # prefetched 3 deep on the same sync DMA queue.
            dep_label = []

            deferred = []

            def drain_bank(g, n, m, pbt):
                prod = spool.tile(
                    [128, 512], f32, tag="prod", name=f"prod_{g}_{n}_{m}"
                )
                nc.vector.scalar_tensor_tensor(
                    out=prod[:],
                    in0=pbt[:],
                    scalar=1.0,
                    in1=dep_label[m][:],
                    op0=Alu.mult,
                    op1=Alu.mult,
                    accum_out=out_sb[m][:, n : n + 1],
                )

            def do_group(g, defer_drain=False):
                n0, n1 = ranges[g]
                wg_sb = wg_tiles[g]
                for li, n in enumerate(range(n0, n1)):
                    for m in range(2):
                        pbt = ps.tile(
                            [128, 512], f32, tag="ps", name=f"pb_{g}_{li}_{m}"
                        )
                        for k in range(4):
                            nc.tensor.matmul(
                                pbt[:],
                                lhsT=selT[k][:, m * 128 : (m + 1) * 128],
                                rhs=wg_sb[
                                    :, (li * 4 + k) * 512 : (li * 4 + k + 1) * 512
                                ],
                                start=(k == 0),
                                stop=(k == 3),
                            )
                        if defer_drain:
                            deferred.append((g, n, m, pbt))
                        else:
                            drain_bank(g, n, m, pbt)
                if g + 4 < N_GROUPS:
                    issue_wg(g + 4)

            do_group(0, defer_drain=True)

            # --- dep projection: dep_label[m] = [128 t, 512 d] f32 --------
            # (fills the PE while the wg stream runs; k 0-3 then 4-7 so the
            # second deppack half can still be in flight)
            for m in range(2):
                pd = ps.tile([128, 512], f32, tag="ps", name=f"pd{m}")
                for k in range(8):
                    h = k // 4
                    nc.tensor.matmul(
                        pd[:],
                        lhsT=dp_sb[
                            :,
                            h * 3072 + 2048 + (k % 4) * 256 + m * 128 : h * 3072
                            + 2048
                            + (k % 4) * 256
                            + m * 128
                            + 128,
                        ],
                        rhs=dp_sb[:, h * 3072 + (k % 4) * 512 : h * 3072 + (k % 4) * 512 + 512],
                        start=(k == 0),
                        stop=(k == 7),
                    )
                dl = pers.tile([128, 512], f32, tag=f"dl{m}", name=f"dl{m}")
                nc.vector.tensor_tensor(
                    dl[:], pd[:], cp_sb[:, C_BDEP : C_BDEP + 512], Alu.add
                )
                dep_label.append(dl)

            # --- flush group-0 drains now that dep_label exists -----------
            for (g, n, m, pbt) in deferred:
                drain_bank(g, n, m, pbt)
            deferred.clear()

            # --- remaining label groups -----------------------------------
            for g in range(1, N_GROUPS):
                do_group(g)

            # --- add label bias, single merged store ----------------------
            fin = pers.tile([128, 128], f32, tag="fin", name="fin")
            for m in range(2):
                nc.vector.tensor_tensor(
                    fin[:, m * 64 : m * 64 + NL],
                    out_sb[m][:, :NL],
                    cp_sb[:, C_BIAS : C_BIAS + NL],
                    Alu.add,
                )
            out_v = out.rearrange("(m p) n -> p m n", m=2)
            fin_v = fin.rearrange("p (m n) -> p m n", m=2)
            nc.sync.dma_start(out_v[:, :, :NL], fin_v[:, :, :NL])

    nc.finalize()
    return nc


def _stage_shared(Wdep, bdep, Whead, bhead, W, bias):
    """Host-side staging of the replicated tensors."""

    def pack_w(Wm):  # [1024, 512] -> [128, 4096] slab-major bf16
        return Wm.reshape(8, 128, 512).transpose(1, 0, 2).reshape(128, 4096)

    whead_h = pack_w(Whead)
    wdep_h = pack_w(Wdep)

    # W[n, d, e] -> WT[n, k, p, d] = W[n, d, k*128+p]
    WT = np.ascontiguousarray(W.transpose(0, 2, 1)).reshape(NL, 4, 128, 512)
    wg_h = np.zeros((N_GROUPS, 128, GF), dtype=BF16)
    for g, (n0, n1) in enumerate(_group_ranges()):
        blk = WT[n0:n1]  # [ng, 4, 128, 512]
        ng = n1 - n0
        wg_h[g, :, : ng * 4 * 512] = (
            blk.transpose(2, 0, 1, 3).reshape(128, ng * 4 * 512).astype(BF16)
        )

    constpack = np.zeros((128, C_TOT), dtype=np.float32)
    constpack[:, C_BDEP : C_BDEP + 512] = bdep[None, :]
    constpack[:, C_BIAS : C_BIAS + NL] = bias[None, :]
    constpack[:, C_IOTA] = np.arange(128, dtype=np.float32)
    constpack[:, C_IOTA + 1] = 128 + np.arange(128, dtype=np.float32)
    constpack[:, C_BHEAD : C_BHEAD + 4] = bhead.reshape(4, 128).T

    return {
        "whead_h": whead_h,
        "wdep_h": wdep_h,
        "wg": wg_h,
        "constpack_base": constpack,
    }


def _pack_proj(w_h, xT_h):
    """[128,4096] weight slabs + [128,2048] activation slabs ->
    [whead k0-3 | headT k0-3 | whead k4-7 | headT k4-7] as [128, 6144]."""
    return np.concatenate(
        [w_h[:, :2048], xT_h[:, :1024], w_h[:, 2048:], xT_h[:, 1024:]], axis=1
    )


def _stage_core(shared, dep_b, head_b, idx_b):
    """Host-side staging of one batch's activations."""

    def pack_x(x):  # [256, 1024] -> [128, 2048] slab-major bf16
        return x.T.reshape(8, 128, 256).transpose(1, 0, 2).reshape(128, 2048)

    headT_h = pack_x(head_b)
    depT_h = pack_x(dep_b)
    headpack = np.ascontiguousarray(
        _pack_proj(shared["whead_h"], headT_h)
    ).astype(BF16)
    deppack = np.ascontiguousarray(_pack_proj(shared["wdep_h"], depT_h)).astype(BF16)
    constpack = shared["constpack_base"].copy()
    constpack[:, C_IDX : C_IDX + 256] = idx_b.astype(np.float32)[None, :]
    return {"headpack": headpack, "deppack": deppack, "constpack": constpack}


def kernel(dep, head, head_indices, mask, Wdep, bdep, Whead, bhead, W, bias):
    global LAST_RESULTS, _NC_CACHE
    from concourse.bass_utils import run_bass_kernel_spmd

    dep = np.asarray(dep, dtype=np.float32)
    head = np.asarray(head, dtype=np.float32)
    head_indices = np.asarray(head_indices)
    Wdep = np.asarray(Wdep, dtype=np.float32)
    bdep = np.asarray(bdep, dtype=np.float32)
    Whead = np.asarray(Whead, dtype=np.float32)
    bhead = np.asarray(bhead, dtype=np.float32)
    W = np.asarray(W, dtype=np.float32)
    bias = np.asarray(bias, dtype=np.float32)

    if _NC_CACHE is None:
        _NC_CACHE = _build_nc()
    nc = _NC_CACHE

    shared = _stage_shared(Wdep, bdep, Whead, bhead, W, bias)
    in_maps = []
    for b in range(B):
        m = {"wg": shared["wg"]}
        m.update(_stage_core(shared, dep[b], head[b], head_indices[b]))
        in_maps.append(m)

    res = run_bass_kernel_spmd(nc, in_maps, list(range(B)))
    LAST_RESULTS = res
    outs = [
        np.asarray(res.results[b]["out"][:, :NL], dtype=np.float32) for b in range(B)
    ]
    return np.stack(outs, axis=0)

